# revision 30
# baseline (speedup 1.0000x reference)
"""Chamfer distance loss kernel for Trainium2 (8 NeuronCores).

Problem: pred_points [4, 8192, 3], gt_points [4, 8192, 3] (f32).
  loss = mean_n min_m ||p_n - g_m|| + mean_m min_n ||p_n - g_m||

Sharding: 8 cores = 4 batches x 2 pred-halves. Each core handles
[4096 pred x 8192 gt] of one batch. Row mins (pred->gt) are complete
per core; col mins (gt->pred) are partial and min-combined on host.

Device math: one K=16 float32r augmented matmul per [128, 512] tile
produces S = 2 p.g - |p|^2 - |g|^2 = -d2 directly in PSUM (signs folded
into the host-packed operands), so every min becomes a max:
  rowout[n] = max_m S[n, m]    (d2 rowmin = -rowout)
  colout[m] = max_n S[n, m]    (partial; host combines halves)
sqrt + means happen on host (32K elements; negligible).

float32r streams at 1 cycle/column (4x faster than fp32) but rounds
inputs to 11 mantissa bits; Dekker-split hi/lo rows restore fp32-class
precision of d2 (see _pack_one). The scalar engine stages each PSUM
group to bf16 SBUF (sole PSUM consumer, keeping PE matmuls at a single
sync wait — this walrus rejects >1 wait per instruction, see
_legalize_waits); the vector engine then runs both min directions at
the bf16 2x tensor_tensor rate. bf16 value rounding is relative, so
the final mean error stays ~1e-5.
"""

import sys

import numpy as np

for _p in ("/opt/trn_rl_repo", "/root/.axon_site/_ro/trn_rl_repo"):
    if _p not in sys.path:
        sys.path.append(_p)

N_CORES = 8
B, N, M, D = 4, 8192, 8192, 3
NP = N // 2  # pred rows per core
# augmented contraction dim: 6 cross piece-pairs x 3 coords + 3 |g|^2 pieces.
# bf16 streams at 1 cycle/column on the PE regardless of K, so the triple
# bf16 Dekker split costs nothing extra; |p|^2 rides the ACT staging bias.
K = 21

_NC_CACHE = {}


def build_nc(np_rows, m_cols, legalize=True):
    """Build the single-core Bass program (same program runs SPMD on all 8)."""
    import concourse.bass as bass
    import concourse.mybir as mybir
    from concourse.tile import TileContext

    f32 = mybir.dt.float32
    f32r = mybir.dt.float32r
    bf16 = mybir.dt.bfloat16
    fmax = mybir.AluOpType.max
    ax_x = mybir.AxisListType.X

    R = np_rows // 128  # pred row-tiles
    G = m_cols // 2048  # groups of 4 psum banks
    KB = m_cols // 32  # 32-col blocks in the transpose tail
    # this toolchain's codegen rejects TensorTensor on the Pool engine, so
    # no GPSIMD offload — everything min-capable runs on the vector engine
    GP_COL = set()

    nc = bass.Bass()
    # single fused input: [:, :np_rows] = predaug, [:, np_rows:] = gtaug,
    # bf16 triple-Dekker-split rows (see _pack_one).
    inaug = nc.declare_dram_parameter("inaug", [K, np_rows + m_cols], bf16,
                                      isOutput=False)
    # -|p|^2 per pred point, applied as the ACT staging bias (fp32 exact)
    p2neg = nc.declare_dram_parameter("p2neg", [128, np_rows // 128], f32,
                                      isOutput=False)
    rowout = nc.declare_dram_parameter("rowout", [np_rows], f32, isOutput=True)
    colout = nc.declare_dram_parameter("colout", [m_cols], f32, isOutput=True)

    with TileContext(nc) as tc:
        with (
            tc.tile_pool(name="persist", bufs=1) as pp,
            tc.tile_pool(name="staged", bufs=10) as st_pool,
            tc.tile_pool(name="roww", bufs=3) as rw_pool,
        ):
            in_sb = pp.tile([K, np_rows + m_cols], bf16)
            nc.sync.dma_start(out=in_sb[:], in_=inaug[:])
            pred_sb = in_sb[:, :np_rows]
            gt_sb = in_sb[:, np_rows:]
            p2_sb = pp.tile([128, R], f32)
            nc.sync.dma_start(out=p2_sb[:], in_=p2neg[:])

            colrun = pp.tile([128, m_cols], bf16)  # running max of S over row-tiles
            rowmaxs = pp.tile([128, R], f32)

            with tc.tile_pool(name="psum", bufs=2, space="PSUM") as psum_pool:
                for r in range(R):
                    staged = []
                    lhsT = pred_sb[:, r * 128 : (r + 1) * 128]
                    for g in range(G):
                        pt = psum_pool.tile([128, 2048], f32)
                        for j in range(4):
                            c = g * 4 + j
                            nc.tensor.matmul(
                                pt[:, j * 512 : (j + 1) * 512],
                                lhsT=lhsT,
                                rhs=gt_sb[:, c * 512 : (c + 1) * 512],
                                start=True,
                                stop=True,
                            )
                        # single PSUM consumer: ACT stages to bf16 SBUF,
                        # folding in the -|p|^2 per-partition bias for free
                        st = st_pool.tile([128, 2048], bf16, tag="staged")
                        nc.scalar.add(out=st[:], in_=pt[:],
                                      add=p2_sb[:, r : r + 1])
                        staged.append(st)
                        # col direction: running elementwise max across row-tiles
                        cslice = colrun[:, g * 2048 : (g + 1) * 2048]
                        eng = nc.gpsimd if g in GP_COL else nc.vector
                        if r == 0:
                            eng.tensor_copy(out=cslice, in_=st[:])
                        else:
                            eng.tensor_tensor(out=cslice, in0=st[:], in1=cslice,
                                              op=fmax)
                    # row direction: fold staged groups with bf16 2x tt-maxes,
                    # then shrink before the (1x-only) reduce. (The native
                    # tensor_tensor_reduce would fuse the last fold + reduce,
                    # but this walrus rejects it: "ISA wrong length".)
                    rw = rw_pool.tile([128, 2048], bf16, tag="rowwide")
                    nc.vector.tensor_tensor(out=rw[:], in0=staged[0][:],
                                            in1=staged[1][:], op=fmax)
                    for g in range(2, G):
                        nc.vector.tensor_tensor(out=rw[:], in0=staged[g][:],
                                                in1=rw[:], op=fmax)
                    nc.vector.tensor_tensor(out=rw[:, 0:1024], in0=rw[:, 0:1024],
                                            in1=rw[:, 1024:2048], op=fmax)
                    nc.vector.tensor_tensor(out=rw[:, 0:512], in0=rw[:, 0:512],
                                            in1=rw[:, 512:1024], op=fmax)
                    nc.vector.tensor_tensor(out=rw[:, 0:256], in0=rw[:, 0:256],
                                            in1=rw[:, 256:512], op=fmax)
                    nc.vector.tensor_reduce(
                        out=rowmaxs[:, r : r + 1], in_=rw[:, 0:256], axis=ax_x,
                        op=fmax,
                    )
                nc.sync.dma_start(
                    out=rowout.rearrange("(r p) -> p r", p=128), in_=rowmaxs[:]
                )

            # tail: cross-partition max of colrun, all on the vector engine.
            # 32x32 stream transpose, reduce each 32-block, fold 4 partition
            # groups, then one strided DMA out.
            colT = pp.tile([128, m_cols], bf16)
            nc.vector.transpose(out=colT[:], in_=colrun[:])
            p4 = pp.tile([128, KB], f32)
            nc.vector.tensor_reduce(
                out=p4[:],
                in_=colT.rearrange("p (k i) -> p k i", i=32),
                axis=ax_x,
                op=fmax,
            )
            # fold the 4 partition groups: DVE lanes are partition-locked, so
            # move groups 1-3 down to partitions 0-31 via SBUF->SBUF DMA first
            scratch = pp.tile([32, 3 * KB], f32)
            for a in (1, 2, 3):
                nc.sync.dma_start(
                    out=scratch[:, (a - 1) * KB : a * KB],
                    in_=p4[32 * a : 32 * (a + 1), :],
                )
            for a in (1, 2, 3):
                nc.vector.tensor_tensor(
                    out=p4[0:32, :],
                    in0=scratch[:, (a - 1) * KB : a * KB],
                    in1=p4[0:32, :],
                    op=fmax,
                )
            nc.sync.dma_start(
                out=colout.rearrange("(k i) -> i k", i=32), in_=p4[0:32, :]
            )

    if legalize:
        _legalize_waits(nc, mybir)
    return nc


def _legalize_waits(nc, mybir):
    """This walrus' codegen accepts only ONE sync wait per instruction.
    Tile emits multi-wait sync_info (e.g. PSUM-reuse matmuls wait on both
    the PE and DVE sems; the tail Drain waits on every proc). Split the
    extras into standalone single-wait EventSemaphore instructions placed
    immediately before the owner on the same engine — same semantics,
    sequential waits."""
    ctr = 0
    for fn in nc.m.functions:
        for bb in fn.blocks:
            out = []
            changed = False
            for ins in bb.instructions:
                si = getattr(ins, "sync_info", None)
                ws = list(si.on_wait) if si is not None and si.on_wait else []
                if len(ws) > 1:
                    changed = True
                    for w in ws[:-1]:
                        ctr += 1
                        out.append(
                            mybir.InstEventSemaphore(
                                name=f"LW-{ctr}",
                                engine=ins.engine,
                                ins=[],
                                outs=[],
                                sync_info=mybir.SyncInfo(on_wait=[w], on_update=[]),
                            )
                        )
                    si.on_wait = [ws[-1]]
                out.append(ins)
            if changed:
                bb.instructions = out


def _split_bf16_3(x):
    """Triple round-to-nearest bf16 split: x ~= x1 + x2 + x3 to ~2^-27."""
    import ml_dtypes

    x = np.ascontiguousarray(x, dtype=np.float32)
    x1 = x.astype(ml_dtypes.bfloat16).astype(np.float32)
    r1 = x - x1
    x2 = r1.astype(ml_dtypes.bfloat16).astype(np.float32)
    r2 = r1 - x2
    x3 = r2.astype(ml_dtypes.bfloat16).astype(np.float32)
    return x1, x2, x3


def _pack_one(p, g):
    """Pack the K=21 bf16 fused operand for one core.

    Device computes S' = 2 p.g - |g|^2 per (n, m) as one K-21 bf16 matmul
    (the -|p|^2 term is the ACT staging bias). With p = p1+p2+p3 and
    g = g1+g2+g3 (bf16 pieces), the kept piece-pairs (1,1) (1,2) (2,1)
    (2,2) (1,3) (3,1) reproduce p.g to ~2^-27; |g|^2 rides as 3 bf16
    pieces against a -1 row. bf16 x bf16 products are exact in the fp32
    PSUM accumulation, so d2 comes out at fp32-class precision.
    """
    import ml_dtypes

    np_rows, m_cols = p.shape[0], g.shape[0]
    p1, p2_, p3 = _split_bf16_3(p.T)  # [3, NP] each
    g1, g2_, g3 = _split_bf16_3(g.T)  # [3, M] each
    gsq = np.sum(g.astype(np.float64) ** 2, axis=-1).astype(np.float32)
    q1, q2, q3 = _split_bf16_3(gsq)

    buf = np.zeros((K, np_rows + m_cols), dtype=np.float32)
    pb, gb = buf[:, :np_rows], buf[:, np_rows:]
    # cross piece-pairs: (pred piece, gt piece)
    pairs = [(p1, g1), (p1, g2_), (p2_, g1), (p2_, g2_), (p1, g3), (p3, g1)]
    for i, (pp_, gp_) in enumerate(pairs):
        pb[3 * i : 3 * i + 3] = 2.0 * pp_
        gb[3 * i : 3 * i + 3] = gp_
    pb[18:21] = -1.0
    gb[18] = q1
    gb[19] = q2
    gb[20] = q3
    return buf.astype(ml_dtypes.bfloat16)


def _pack_p2neg(p):
    """[128, R] f32: entry [lane, r] = -|p[r*128+lane]|^2."""
    psq = np.sum(p.astype(np.float64) ** 2, axis=-1).astype(np.float32)
    return np.ascontiguousarray(-psq.reshape(-1, 128).T)


def _pack_inputs(pred_points, gt_points):
    pred_points = np.asarray(pred_points, dtype=np.float32)
    gt_points = np.asarray(gt_points, dtype=np.float32)
    in_maps = []
    for k in range(N_CORES):
        b, h = k // 2, k % 2
        p = pred_points[b, h * NP : (h + 1) * NP]  # [NP, 3]
        g = gt_points[b]  # [M, 3]
        in_maps.append({"inaug": _pack_one(p, g), "p2neg": _pack_p2neg(p)})
    return in_maps


_LDW_OPT_PATCHED = False


def _enable_ldw_opt():
    """bass_utils hardcodes --enable-ldw-opt=false in the walrus command.
    16 consecutive matmuls here share one stationary operand, so LDWEIGHTS
    CSE removes ~30% of PE busy time. Rewrite the flag via a run_command
    shim."""
    global _LDW_OPT_PATCHED
    if _LDW_OPT_PATCHED:
        return
    from concourse import bass_utils

    orig = bass_utils.run_command

    def patched(cmd, *args, **kwargs):
        if isinstance(cmd, list):
            cmd = [
                "--enable-ldw-opt=true" if c == "--enable-ldw-opt=false" else c
                for c in cmd
            ]
        return orig(cmd, *args, **kwargs)

    bass_utils.run_command = patched
    _LDW_OPT_PATCHED = True


def run_on_cores(in_maps, trace=False):
    from concourse.bass_utils import run_bass_kernel_spmd

    # ldw-opt stays OFF: walrus rejects the standalone bf16 InstLdweights
    # under LDW optimization ("not compatible"), and bf16 weight loads are
    # FWL-eligible and cheap anyway.

    key = (NP, M)
    if key not in _NC_CACHE:
        _NC_CACHE[key] = build_nc(NP, M)
    nc = _NC_CACHE[key]
    return run_bass_kernel_spmd(nc, in_maps, list(range(N_CORES)), trace=trace)


def _combine(results):
    """Host reduction: negate (S = -d2), combine halves, sqrt, means."""
    total = np.float64(0.0)
    for b in range(B):
        r0, r1 = results[2 * b], results[2 * b + 1]
        row_d2 = -np.concatenate([r0["rowout"], r1["rowout"]])  # [N]
        col_d2 = np.minimum(-r0["colout"], -r1["colout"])  # [M]
        total += np.sqrt(np.maximum(row_d2, 0.0), dtype=np.float64).mean() / B
        total += np.sqrt(np.maximum(col_d2, 0.0), dtype=np.float64).mean() / B
    return np.float32(total)


def kernel(pred_points, gt_points):
    in_maps = _pack_inputs(pred_points, gt_points)
    res = run_on_cores(in_maps)
    return _combine(res.results)


if __name__ == "__main__":
    rng = np.random.default_rng(0)
    pred = rng.standard_normal((B, N, D), dtype=np.float32)
    gt = rng.standard_normal((B, M, D), dtype=np.float32)
    out = kernel(pred, gt)
    print("kernel output:", out)
